# revision 33
# baseline (speedup 1.0000x reference)
"""Trainium2 Bass kernel for a character-CNN word encoder.

Computation (per word of W=20 chars):
  x = emb[chars]                       # [W, E=64] -> [E, W]
  y_k = conv1d(x, w_k, 'same') + b_k   # k in {1,3,5}, H=256 channels
  m_k = max_t relu(y_k)                # [H]
  out = concat(m1, m3, m5) @ lw.T + lb # [H]

Strategy (pure data parallel over N = B*S = 8192 words, 1024 words/core):
  - Embedding gather via SBUF-source dma_gather(transpose=True): the 257-row
    bf16 table lives in SBUF with tokens_per_rank=128 / 256 B per rank, so
    the TX side is one 256-byte descriptor per index (same packet shape as
    the HBM-source path). The index stream inserts 2 pad tokens around each
    word so the gather materializes X with 'same'-conv zero padding built in.
  - Gathers are split per half-chunk (1408 idxs) and spread round-robin over
    4 SWDGE queues: queue q is served by Q7 core pair (2q, 2q+1), so
    descriptor generation parallelizes 4x. A tiny warmup gather triggers the
    one-time Q7 IRAM library load before the real inputs land.
  - A SBUF->SBUF DMA writes rows 64:128 with X shifted left by one column,
    so a K=128 matmul contracts TWO consecutive conv taps at once.
  - Convs per 16-word block: 6 full K=128 tap-pair matmuls + 3 concurrent
    pairs of K=64 row-tiled matmuls for the odd taps (tile_position rows
    0/64) -> 9 matmul slots for 18 taps = 100% PE-array utilization.
  - Drain/max: ACT copies PSUM cols c[4:20] -> bf16 SBUF, DVE copies c[0:4];
    a per-chunk tensor_tensor(max) tree in bf16 SBUF (2x DVE mode) reduces
    20 -> 1; the last tiny level and the bias+relu run on GPSIMD to balance
    engines.
  - Linear layer: 6 K=128 matmuls (m as stationary) + a K=1 ones-row matmul
    that adds lb. Output lands as [words, 256] fp32, DMAed contiguously.
"""

import numpy as np
import ml_dtypes

import concourse.bass as bass
import concourse.tile as tile
import concourse.mybir as mybir
from concourse import bacc
from concourse.bass_utils import run_bass_kernel_spmd

BF16 = ml_dtypes.bfloat16

# Problem shape (hardcoded per contest rules).
B, S, W = 64, 128, 20
VOCAB, E, H = 256, 64, 256
N_CORES = 8
NW = (B * S) // N_CORES       # words per core = 1024
WP = 22                       # word frame: [z z t0..t19]; right pads are the
                              # NEXT word's left pads (halo zeros at chunk end)
PAD_TOK = VOCAB               # index of the all-zero table row
CHUNK_W = 128                 # words per gather chunk
N_CHUNKS = NW // CHUNK_W      # 8
NB = 16                       # words per matmul block
N_BLOCKS = CHUNK_W // NB      # 8
IDX_PER_CHUNK = CHUNK_W * WP      # 2816 tokens per chunk
IDX_COLS = NW * WP // 16          # 1408 (token t at idx[t % 16, t // 16])
HG = IDX_PER_CHUNK // 2           # 1408 idxs per half-chunk gather

# Conv matmul plan.
# Full K=128 blocks (conv k, half h, first tap d): contract taps (d, d+1)
# via the shifted rows 64:128.
FULLS = [(5, 0, 0), (5, 1, 0), (5, 0, 2), (5, 1, 2), (3, 0, 0), (3, 1, 0)]
# Packed K=64 blocks (conv k, last tap d): rows 0:64 = h0, rows 64:128 = h1,
# run as two concurrent row-tiled matmuls (tile_position rows 0 / 64).
PACKS = [(5, 4), (3, 2), (1, 0)]
NQ = len(FULLS) + len(PACKS)  # 9 weight blocks of [128, 128]

# Region r = (conv,half): 0=c5h0 1=c5h1 2=c3h0 3=c3h1 4=c1h0 5=c1h1.
# Tile A holds regions 0-2, tile B regions 3-5 (slot = r % 3).
REGION = {(5, 0): 0, (5, 1): 1, (3, 0): 2, (3, 1): 3, (1, 0): 4, (1, 1): 5}

# lw column ranges per region (reference concat order: conv1, conv3, conv5).
LW_COLS = {0: (512, 640), 1: (640, 768), 2: (256, 384), 3: (384, 512),
           4: (0, 128), 5: (128, 256)}


def _build_nc():
    f32 = mybir.dt.float32
    bf16 = mybir.dt.bfloat16
    i16 = mybir.dt.int16
    AF = mybir.ActivationFunctionType
    ALU = mybir.AluOpType

    nc = bacc.Bacc("TRN2", target_bir_lowering=False, debug=False,
                   num_swdge_queues=4)

    idx_d = nc.dram_tensor("idx", [128, IDX_COLS], i16, kind="ExternalInput").ap()
    # Embedding table for SBUF-source dma_gather with tokens_per_rank=128 /
    # free_dim_per_rank=256 B: token v's whole 256-byte row lives in
    # partition v & 127 at byte range [(v>>7)*256, +256).
    table_d = nc.dram_tensor("table", [128, 3 * 128], bf16, kind="ExternalInput").ap()
    wconv_d = nc.dram_tensor("wconv", [128, NQ * 128], bf16, kind="ExternalInput").ap()
    lwt_d = nc.dram_tensor("lwt", [128, 6 * 256], bf16, kind="ExternalInput").ap()
    cbias_d = nc.dram_tensor("cbias", [128, 6], f32, kind="ExternalInput").ap()
    lbias_d = nc.dram_tensor("lbias", [1, 256], bf16, kind="ExternalInput").ap()
    out_d = nc.dram_tensor("out", [NW, H], f32, kind="ExternalOutput").ap()

    with tile.TileContext(nc) as tc:
        with (
            tc.tile_pool(name="consts", bufs=1) as cpool,
            tc.tile_pool(name="xx", bufs=4) as xxpool,
            tc.tile_pool(name="ybuf", bufs=2) as ybpool,
            tc.tile_pool(name="tree", bufs=2) as tpool,
            tc.tile_pool(name="mtile", bufs=2) as mpool,
            tc.tile_pool(name="osb", bufs=2) as opool,
            tc.tile_pool(name="psA", bufs=2, space="PSUM") as psa_pool,
            tc.tile_pool(name="psO", bufs=2, space="PSUM") as pso_pool,
        ):
            # --- Q7 gather-library warmup: a throwaway gather whose inputs
            # are memset-ready immediately, so the one-time IRAM load overlaps
            # the real input DMAs instead of serializing after them.
            wu_idx = cpool.tile([128, 8], i16, tag="wuidx")
            nc.vector.memset(wu_idx[:], 0)
            wu_tab = cpool.tile([128, 128], bf16, tag="wutab")
            nc.vector.memset(wu_tab[:], 0.0)
            wu_out = cpool.tile([128, 128], bf16, tag="wuout")
            nc.gpsimd.dma_gather(
                wu_out[:].rearrange("p (a n) -> p a n", a=1),
                wu_tab[:],
                wu_idx[:],
                num_idxs=128,
                num_idxs_reg=128,
                elem_size=128,
                transpose=True,
                single_packet=False,
                queue_num=0,
                sbuf_tokens_per_rank=128,
                sbuf_free_dim_per_rank=256,
            )

            # --- constants (idx first: it alone gates the gathers) ---
            idx_sb = cpool.tile([128, IDX_COLS], i16, tag="idx")
            nc.sync.dma_start(idx_sb[:], idx_d[:])
            table_sb = cpool.tile([128, 3 * 128], bf16, tag="table")
            nc.sync.dma_start(table_sb[:], table_d[:])
            wconv_sb = cpool.tile([128, NQ * 128], bf16, tag="wconv")
            nc.sync.dma_start(wconv_sb[:], wconv_d[:])
            lwt_sb = cpool.tile([128, 6 * 256], bf16, tag="lwt")
            nc.sync.dma_start(lwt_sb[:], lwt_d[:])
            cbias_sb = cpool.tile([128, 6], f32, tag="cbias")
            nc.sync.dma_start(cbias_sb[:], cbias_d[:])
            lbias_sb = cpool.tile([1, 256], bf16, tag="lbias")
            nc.sync.dma_start(lbias_sb[:], lbias_d[:])
            ones_sb = cpool.tile([1, 128], bf16, tag="ones")
            nc.vector.memset(ones_sb[:], 1.0)

            # --- gather pipeline: queue all chunks up front; two half-chunk
            # gathers per chunk, round-robin over the 4 SWDGE queues so all
            # four Q7 core pairs generate descriptors in parallel.
            # Tile assigns DMASW sem lanes to Pool DMAs round-robin over 8
            # lanes, and each sem lane is locked to one SWDGE queue; with
            # queue = (pool-DMA index) % 4 (warmup = index 0) the lane<->queue
            # mapping stays consistent.
            xx_tiles = []
            gq = 1
            for c in range(N_CHUNKS):
                xx = xxpool.tile([128, IDX_PER_CHUNK + 8], bf16, tag="xx")
                for h in range(2):
                    nc.gpsimd.dma_gather(
                        xx[:, h * HG:(h + 1) * HG].rearrange("p (a n) -> p a n", a=1),
                        table_sb[:],
                        idx_sb[:, (c * IDX_PER_CHUNK + h * HG) // 16:
                               (c * IDX_PER_CHUNK + (h + 1) * HG) // 16],
                        num_idxs=HG,
                        num_idxs_reg=HG,
                        elem_size=128,
                        transpose=True,
                        single_packet=False,
                        queue_num=0,
                        sbuf_tokens_per_rank=128,
                        sbuf_free_dim_per_rank=256,
                    )
                    gq += 1
                nc.vector.memset(xx[:, IDX_PER_CHUNK:], 0.0)
                # rows 64:128 = rows 0:64 shifted left one column
                nc.sync.dma_start(
                    xx[64:128, 0:IDX_PER_CHUNK], xx[0:64, 1:IDX_PER_CHUNK + 1]
                )
                xx_tiles.append(xx)

            def emit_linear(ci, m_all):
                # linear layer: out[words, 256] = m.T @ lwT + lb
                op = pso_pool.tile([128, 256], f32, tag="psO")
                for r in range(6):
                    nc.tensor.matmul(
                        op[:], lhsT=m_all[:, r, :],
                        rhs=lwt_sb[:, r * 256:(r + 1) * 256],
                        start=(r == 0), stop=False,
                    )
                nc.tensor.matmul(
                    op[:], lhsT=ones_sb[0:1, :], rhs=lbias_sb[0:1, :],
                    start=False, stop=True,
                )
                osb = opool.tile([128, 256], f32, tag="osb")
                nc.vector.tensor_copy(out=osb[:], in_=op[:])
                nc.sync.dma_start(out_d[ci * CHUNK_W:(ci + 1) * CHUNK_W, :], osb[:])

            m_prev = None
            for c in range(N_CHUNKS):
                xx = xx_tiles[c]
                # bf16 staging of conv outputs: ybu = cols c[4:20], yc = c[0:4]
                ybu = ybpool.tile([128, 6, CHUNK_W, 16], bf16, tag="ybu")
                yc = ybpool.tile([128, 6, CHUNK_W, 4], bf16, tag="yc")

                for b in range(N_BLOCKS):
                    base = b * NB * WP

                    def rhs_at(rows, j0):
                        lo, hi = rows
                        return (
                            xx[lo:hi, base + j0: base + j0 + NB * WP]
                            .rearrange("p (w c) -> p w c", c=WP)[:, :, 0:W]
                        )

                    ps_a = psa_pool.tile([128, 1536], f32, tag="psA")
                    ps_b = psa_pool.tile([128, 1536], f32, tag="psA")

                    def ps_slot(r):
                        ps = ps_a if r < 3 else ps_b
                        s = r % 3
                        return ps[:, s * 512: s * 512 + NB * W]

                    # Full K=128 tap-pair matmuls (start of each region group).
                    for q, (k, hh, d) in enumerate(FULLS):
                        r = REGION[(k, hh)]
                        pad = k // 2
                        j0 = d - pad + 2
                        nc.tensor.matmul(
                            ps_slot(r),
                            lhsT=wconv_sb[:, q * 128:(q + 1) * 128],
                            rhs=rhs_at((0, 128), j0),
                            start=(d == 0),
                            stop=False,
                        )
                    # Packed singleton taps: per conv, h0 on rows 0:64 and h1
                    # on rows 64:128 run as concurrent row-tiled matmuls into
                    # different PSUM banks.
                    for pi, (k, d) in enumerate(PACKS):
                        q = len(FULLS) + pi
                        pad = k // 2
                        j0 = d - pad + 2
                        st = (k == 1)  # conv1 is a single-matmul group
                        nc.tensor.matmul(
                            ps_slot(REGION[(k, 0)]),
                            lhsT=wconv_sb[0:64, q * 128:(q + 1) * 128],
                            rhs=rhs_at((0, 64), j0),
                            start=st,
                            stop=True,
                        )
                        nc.tensor.matmul(
                            ps_slot(REGION[(k, 1)]),
                            lhsT=wconv_sb[64:128, q * 128:(q + 1) * 128],
                            rhs=rhs_at((64, 128), j0 - 1),
                            start=st,
                            stop=True,
                        )

                    # Parallel drains to bf16 SBUF: ACT takes c[4:20],
                    # DVE takes c[0:4].
                    for r0, ps in ((0, ps_a), (3, ps_b)):
                        pv = (
                            ps[:, 0:1536]
                            .rearrange("p (r b) -> p r b", b=512)[:, :, 0:NB * W]
                            .rearrange("p r (w c) -> p r w c", c=W)
                        )
                        nc.scalar.copy(
                            out=ybu[:, r0:r0 + 3, b * NB:(b + 1) * NB, :],
                            in_=pv[:, :, :, 4:20],
                        )
                        nc.vector.tensor_copy(
                            out=yc[:, r0:r0 + 3, b * NB:(b + 1) * NB, :],
                            in_=pv[:, :, :, 0:4],
                        )

                # Max tree in bf16 SBUF (DVE 2x mode; any pairing of positions
                # is valid for a max). Two half-chunk trees so the first
                # half's tree overlaps the second half's PSUM drains, and
                # bias+relu on DVE tensor_scalar (4x) to keep the ACT queue
                # free for the next chunk's drains.
                m_all = mpool.tile([128, 6, CHUNK_W], bf16, tag="m_all")
                HW2 = CHUNK_W // 2
                for hf in range(2):
                    w0 = hf * HW2
                    ybv = ybu[:, :, w0:w0 + HW2, :]
                    u1 = tpool.tile([128, 6, HW2, 8], bf16, tag="u1")
                    nc.vector.tensor_tensor(out=u1[:], in0=ybv[:, :, :, 0:8],
                                            in1=ybv[:, :, :, 8:16], op=ALU.max)
                    u2 = tpool.tile([128, 6, HW2, 4], bf16, tag="u2")
                    nc.vector.tensor_tensor(out=u2[:], in0=u1[:, :, :, 0:4],
                                            in1=u1[:, :, :, 4:8], op=ALU.max)
                    u3 = tpool.tile([128, 6, HW2, 4], bf16, tag="u3")
                    nc.vector.tensor_tensor(out=u3[:], in0=u2[:],
                                            in1=yc[:, :, w0:w0 + HW2, :],
                                            op=ALU.max)
                    u4 = tpool.tile([128, 6, HW2, 2], bf16, tag="u4")
                    nc.vector.tensor_tensor(out=u4[:], in0=u3[:, :, :, 0:2],
                                            in1=u3[:, :, :, 2:4], op=ALU.max)
                    m_pre = tpool.tile([128, 6, HW2], bf16, tag="m_pre")
                    nc.vector.tensor_tensor(out=m_pre[:], in0=u4[:, :, :, 0],
                                            in1=u4[:, :, :, 1], op=ALU.max)
                    for r in range(6):
                        nc.vector.tensor_scalar(
                            out=m_all[:, r, w0:w0 + HW2],
                            in0=m_pre[:, r, :],
                            scalar1=cbias_sb[:, r:r + 1],
                            scalar2=0.0,
                            op0=ALU.add,
                            op1=ALU.max,
                        )

                # Defer the linear layer by one chunk: chunk c-1's m_all is
                # long since ready, so the PE never stalls on the DVE tree /
                # ACT bias chain (its FIFO otherwise parks on this chunk's
                # linear matmuls while the next chunk's convs are ready).
                if m_prev is not None:
                    emit_linear(c - 1, m_prev)
                m_prev = m_all

            emit_linear(N_CHUNKS - 1, m_prev)

    nc.compile()
    return nc


def _prep_maps(chars, emb, w1, b1, w3, b3, w5, b5, lw, lb):
    flat = np.asarray(chars).reshape(-1, W).astype(np.int64)  # [8192, 20]
    emb = np.asarray(emb, dtype=np.float32)
    lw = np.asarray(lw, dtype=np.float32)
    convs = {1: np.asarray(w1, np.float32), 3: np.asarray(w3, np.float32),
             5: np.asarray(w5, np.float32)}
    biases = {1: np.asarray(b1, np.float32), 3: np.asarray(b3, np.float32),
              5: np.asarray(b5, np.float32)}

    rows = np.zeros((VOCAB + 1, 128), dtype=BF16)
    rows[:VOCAB, :E] = emb.astype(BF16)
    # SBUF-source gather layout (tpr=128, 256 B per rank): token v's row at
    # partition v & 127, bf16 elems [(v>>7)*128, +128).
    table = np.zeros((128, 3 * 128), dtype=BF16)
    for v in range(VOCAB + 1):
        r, tok = v >> 7, v & 127
        table[tok, r * 128:(r + 1) * 128] = rows[v]

    wconv = np.zeros((128, NQ * 128), dtype=BF16)
    for q, (k, hh, d) in enumerate(FULLS):
        wk = convs[k]  # [H, E, k]
        blk = np.zeros((128, 128), dtype=np.float32)
        blk[:E, :] = wk[hh * 128:(hh + 1) * 128, :, d].T
        blk[E:, :] = wk[hh * 128:(hh + 1) * 128, :, d + 1].T
        wconv[:, q * 128:(q + 1) * 128] = blk.astype(BF16)
    for pi, (k, d) in enumerate(PACKS):
        q = len(FULLS) + pi
        wk = convs[k]
        blk = np.zeros((128, 128), dtype=np.float32)
        blk[:E, :] = wk[0:128, :, d].T        # h0 channels, rows 0:64
        blk[E:, :] = wk[128:256, :, d].T      # h1 channels, rows 64:128
        wconv[:, q * 128:(q + 1) * 128] = blk.astype(BF16)

    lwt = np.zeros((128, 6 * 256), dtype=BF16)
    for r in range(6):
        lo, hi = LW_COLS[r]
        lwt[:, r * 256:(r + 1) * 256] = lw[:, lo:hi].T.astype(BF16)

    cbias = np.zeros((128, 6), dtype=np.float32)
    for r, (k, half) in enumerate([(5, 0), (5, 1), (3, 0), (3, 1), (1, 0), (1, 1)]):
        cbias[:, r] = biases[k][half * 128:(half + 1) * 128]

    lbias = np.asarray(lb, np.float32).reshape(1, 256).astype(BF16)

    in_maps = []
    for c in range(N_CORES):
        words = flat[c * NW:(c + 1) * NW]  # [NW, 20]
        padded = np.full((NW, WP), PAD_TOK, dtype=np.int16)
        padded[:, 2:2 + W] = words
        stream = padded.reshape(-1)  # [NW*22]
        wrapped = stream.reshape(-1, 16).T  # [16, IDX_COLS]
        idx = np.ascontiguousarray(np.tile(wrapped, (8, 1)))  # replicated x8
        in_maps.append({
            "idx": idx, "table": table, "wconv": wconv, "lwt": lwt,
            "cbias": cbias, "lbias": lbias,
        })
    return in_maps


_NC_CACHE = {}


def run(inputs, trace=False):
    if "nc" not in _NC_CACHE:
        _NC_CACHE["nc"] = _build_nc()
    nc = _NC_CACHE["nc"]
    in_maps = _prep_maps(**inputs)
    res = run_bass_kernel_spmd(nc, in_maps, list(range(N_CORES)), trace=trace)
    out = np.concatenate([res.results[i]["out"] for i in range(N_CORES)], axis=0)
    return out.reshape(B, S, H).astype(np.float32), res


def kernel(**inputs):
    out, _ = run(inputs)
    return out


# revision 38
# speedup vs baseline: 1.2139x; 1.2139x over previous
"""Trainium2 Bass kernel for a character-CNN word encoder.

Computation (per word of W=20 chars):
  x = emb[chars]                       # [W, E=64] -> [E, W]
  y_k = conv1d(x, w_k, 'same') + b_k   # k in {1,3,5}, H=256 channels
  m_k = max_t relu(y_k)                # [H]
  out = concat(m1, m3, m5) @ lw.T + lb # [H]

Strategy (pure data parallel over N = B*S = 8192 words, 1024 words/core):
  - Embedding gather via SBUF-source dma_gather(transpose=True): the 257-row
    bf16 table lives in SBUF with tokens_per_rank=128 / 256 B per rank, so
    the TX side is one 256-byte descriptor per index (same packet shape as
    the HBM-source path). The index stream inserts 2 pad tokens around each
    word so the gather materializes X with 'same'-conv zero padding built in.
  - Gathers are split per half-chunk (1408 idxs), all on SWDGE queue 0: the
    queue selects the serving Q7 core pair, but concurrent transpose-mode
    gathers on multiple queues interleave packets through the shared XBAR
    and corrupt data on HW, so they must stay serialized on one queue. A
    tiny warmup gather triggers the one-time Q7 IRAM library load early.
  - A SBUF->SBUF DMA writes rows 64:128 with X shifted left by one column,
    so a K=128 matmul contracts TWO consecutive conv taps at once. 12
    tap-pair matmuls per 16-word block (odd taps zero-pad their second
    half; K=64 row-tiled packing is correct on HW but slower here because
    ldw-opt is disabled and 18 instructions pay 18 serial LDWEIGHTS).
  - Drain/max: ACT copies PSUM cols c[4:20] -> bf16 SBUF, DVE copies c[0:4];
    per-half-chunk tensor_tensor(max) trees in bf16 SBUF (2x DVE mode)
    reduce 20 -> 1, overlapping the other half's drains; bias+relu fuses
    into one DVE tensor_scalar(add, max) per region so the ACT queue stays
    free for the next chunk's drains (keeps the PE warm across chunks).
  - Linear layer: 6 K=128 matmuls (m as stationary) + a K=1 ones-row matmul
    that adds lb, deferred by one chunk so the PE never parks on the DVE
    tree. Output lands as [words, 256] fp32, DMAed contiguously.
"""

import numpy as np
import ml_dtypes

import concourse.bass as bass
import concourse.tile as tile
import concourse.mybir as mybir
from concourse import bacc
from concourse.bass_utils import run_bass_kernel_spmd

BF16 = ml_dtypes.bfloat16

# Problem shape (hardcoded per contest rules).
B, S, W = 64, 128, 20
VOCAB, E, H = 256, 64, 256
N_CORES = 8
NW = (B * S) // N_CORES       # words per core = 1024
WP = 22                       # word frame: [z z t0..t19]; right pads are the
                              # NEXT word's left pads (halo zeros at chunk end)
PAD_TOK = VOCAB               # index of the all-zero table row
CHUNK_W = 128                 # words per gather chunk
N_CHUNKS = NW // CHUNK_W      # 8
# Matmul block sizes in words: N = wn*20 rhs columns per matmul; 25 words
# -> N=500, just under the 512-fp32 PSUM bank limit. Fewer, larger matmuls
# matter because ldw-opt is disabled (each instruction pays ~100ns serial
# LDWEIGHTS), so 72 conv matmuls/chunk beat 96.
W_SIZES = [25, 25, 25, 25, 25, 3]
assert sum(W_SIZES) == CHUNK_W
IDX_PER_CHUNK = CHUNK_W * WP      # 2816 tokens per chunk
IDX_COLS = NW * WP // 16          # 1408 (token t at idx[t % 16, t // 16])
HG = IDX_PER_CHUNK // 2           # 1408 idxs per half-chunk gather

# Conv matmul plan: K=128 tap-pair blocks (conv k, half h, first tap d).
# Taps (d, d+1) contract in one matmul via the shifted rows 64:128; when
# d+1 >= k the second half of the weight block is zero.
TAPS = [(5, 0, 0), (5, 1, 0), (5, 0, 2), (5, 1, 2), (5, 0, 4), (5, 1, 4),
        (3, 0, 0), (3, 1, 0), (3, 0, 2), (3, 1, 2), (1, 0, 0), (1, 1, 0)]
NQ = len(TAPS)  # 12 weight blocks of [128, 128]

# Region r = (conv,half): 0=c5h0 1=c5h1 2=c3h0 3=c3h1 4=c1h0 5=c1h1.
# Tile A holds regions 0-2, tile B regions 3-5 (slot = r % 3).
REGION = {(5, 0): 0, (5, 1): 1, (3, 0): 2, (3, 1): 3, (1, 0): 4, (1, 1): 5}

# lw column ranges per region (reference concat order: conv1, conv3, conv5).
LW_COLS = {0: (512, 640), 1: (640, 768), 2: (256, 384), 3: (384, 512),
           4: (0, 128), 5: (128, 256)}


def _build_nc():
    f32 = mybir.dt.float32
    bf16 = mybir.dt.bfloat16
    i16 = mybir.dt.int16
    AF = mybir.ActivationFunctionType
    ALU = mybir.AluOpType

    nc = bacc.Bacc("TRN2", target_bir_lowering=False, debug=False,
                   num_swdge_queues=4)

    idx_d = nc.dram_tensor("idx", [128, IDX_COLS], i16, kind="ExternalInput").ap()
    # Embedding table for SBUF-source dma_gather with tokens_per_rank=128 /
    # free_dim_per_rank=256 B: token v's whole 256-byte row lives in
    # partition v & 127 at byte range [(v>>7)*256, +256).
    table_d = nc.dram_tensor("table", [128, 3 * 128], bf16, kind="ExternalInput").ap()
    wconv_d = nc.dram_tensor("wconv", [128, NQ * 128], bf16, kind="ExternalInput").ap()
    lwt_d = nc.dram_tensor("lwt", [128, 6 * 256], bf16, kind="ExternalInput").ap()
    cbias_d = nc.dram_tensor("cbias", [128, 6], f32, kind="ExternalInput").ap()
    lbias_d = nc.dram_tensor("lbias", [1, 256], bf16, kind="ExternalInput").ap()
    out_d = nc.dram_tensor("out", [NW, H], f32, kind="ExternalOutput").ap()

    with tile.TileContext(nc) as tc:
        with (
            tc.tile_pool(name="consts", bufs=1) as cpool,
            tc.tile_pool(name="xx", bufs=4) as xxpool,
            tc.tile_pool(name="ybuf", bufs=2) as ybpool,
            tc.tile_pool(name="tree", bufs=2) as tpool,
            tc.tile_pool(name="mtile", bufs=2) as mpool,
            tc.tile_pool(name="osb", bufs=2) as opool,
            tc.tile_pool(name="psA", bufs=2, space="PSUM") as psa_pool,
            tc.tile_pool(name="psO", bufs=2, space="PSUM") as pso_pool,
        ):
            # --- Q7 gather-library warmup: a throwaway gather whose inputs
            # are memset-ready immediately, so the one-time IRAM load overlaps
            # the real input DMAs instead of serializing after them.
            wu_idx = cpool.tile([128, 8], i16, tag="wuidx")
            nc.vector.memset(wu_idx[:], 0)
            wu_tab = cpool.tile([128, 128], bf16, tag="wutab")
            nc.vector.memset(wu_tab[:], 0.0)
            wu_out = cpool.tile([128, 128], bf16, tag="wuout")
            nc.gpsimd.dma_gather(
                wu_out[:].rearrange("p (a n) -> p a n", a=1),
                wu_tab[:],
                wu_idx[:],
                num_idxs=128,
                num_idxs_reg=128,
                elem_size=128,
                transpose=True,
                single_packet=False,
                queue_num=0,
                sbuf_tokens_per_rank=128,
                sbuf_free_dim_per_rank=256,
            )

            # --- constants (idx first: it alone gates the gathers) ---
            idx_sb = cpool.tile([128, IDX_COLS], i16, tag="idx")
            nc.sync.dma_start(idx_sb[:], idx_d[:])
            table_sb = cpool.tile([128, 3 * 128], bf16, tag="table")
            nc.sync.dma_start(table_sb[:], table_d[:])
            wconv_sb = cpool.tile([128, NQ * 128], bf16, tag="wconv")
            nc.sync.dma_start(wconv_sb[:], wconv_d[:])
            lwt_sb = cpool.tile([128, 6 * 256], bf16, tag="lwt")
            nc.sync.dma_start(lwt_sb[:], lwt_d[:])
            cbias_sb = cpool.tile([128, 6], f32, tag="cbias")
            nc.sync.dma_start(cbias_sb[:], cbias_d[:])
            lbias_sb = cpool.tile([1, 256], bf16, tag="lbias")
            nc.sync.dma_start(lbias_sb[:], lbias_d[:])
            ones_sb = cpool.tile([1, 128], bf16, tag="ones")
            nc.vector.memset(ones_sb[:], 1.0)

            # --- gather pipeline: queue all chunks up front; two half-chunk
            # gathers per chunk, round-robin over the 4 SWDGE queues so all
            # four Q7 core pairs generate descriptors in parallel.
            # Tile assigns DMASW sem lanes to Pool DMAs round-robin over 8
            # lanes, and each sem lane is locked to one SWDGE queue; with
            # queue = (pool-DMA index) % 4 (warmup = index 0) the lane<->queue
            # mapping stays consistent.
            xx_tiles = []
            gq = 1
            for c in range(N_CHUNKS):
                xx = xxpool.tile([128, IDX_PER_CHUNK + 8], bf16, tag="xx")
                for h in range(2):
                    nc.gpsimd.dma_gather(
                        xx[:, h * HG:(h + 1) * HG].rearrange("p (a n) -> p a n", a=1),
                        table_sb[:],
                        idx_sb[:, (c * IDX_PER_CHUNK + h * HG) // 16:
                               (c * IDX_PER_CHUNK + (h + 1) * HG) // 16],
                        num_idxs=HG,
                        num_idxs_reg=HG,
                        elem_size=128,
                        transpose=True,
                        single_packet=False,
                        queue_num=0,
                        sbuf_tokens_per_rank=128,
                        sbuf_free_dim_per_rank=256,
                    )
                    gq += 1
                nc.vector.memset(xx[:, IDX_PER_CHUNK:], 0.0)
                # rows 64:128 = rows 0:64 shifted left one column
                nc.sync.dma_start(
                    xx[64:128, 0:IDX_PER_CHUNK], xx[0:64, 1:IDX_PER_CHUNK + 1]
                )
                xx_tiles.append(xx)

            def emit_linear(ci, m_all):
                # linear layer: out[words, 256] = m.T @ lwT + lb
                op = pso_pool.tile([128, 256], f32, tag="psO")
                for r in range(6):
                    nc.tensor.matmul(
                        op[:], lhsT=m_all[:, r, :],
                        rhs=lwt_sb[:, r * 256:(r + 1) * 256],
                        start=(r == 0), stop=False,
                    )
                nc.tensor.matmul(
                    op[:], lhsT=ones_sb[0:1, :], rhs=lbias_sb[0:1, :],
                    start=False, stop=True,
                )
                osb = opool.tile([128, 256], f32, tag="osb")
                nc.vector.tensor_copy(out=osb[:], in_=op[:])
                nc.sync.dma_start(out_d[ci * CHUNK_W:(ci + 1) * CHUNK_W, :], osb[:])

            m_prev = None
            for c in range(N_CHUNKS):
                xx = xx_tiles[c]
                # bf16 staging of conv outputs: ybu = cols c[4:20], yc = c[0:4]
                ybu = ybpool.tile([128, 6, CHUNK_W, 16], bf16, tag="ybu")
                yc = ybpool.tile([128, 6, CHUNK_W, 4], bf16, tag="yc")

                w0 = 0
                for wn in W_SIZES:
                    base = w0 * WP

                    def rhs_at(rows, j0):
                        lo, hi = rows
                        return (
                            xx[lo:hi, base + j0: base + j0 + wn * WP]
                            .rearrange("p (w c) -> p w c", c=WP)[:, :, 0:W]
                        )

                    ps_a = psa_pool.tile([128, 1536], f32, tag="psA")
                    ps_b = psa_pool.tile([128, 1536], f32, tag="psA")

                    def ps_slot(r):
                        ps = ps_a if r < 3 else ps_b
                        s = r % 3
                        return ps[:, s * 512: s * 512 + wn * W]

                    # K=128 tap-pair matmuls.
                    for q, (k, hh, d) in enumerate(TAPS):
                        r = REGION[(k, hh)]
                        pad = k // 2
                        j0 = d - pad + 2
                        nc.tensor.matmul(
                            ps_slot(r),
                            lhsT=wconv_sb[:, q * 128:(q + 1) * 128],
                            rhs=rhs_at((0, 128), j0),
                            start=(d == 0),
                            stop=(d + 2 >= k),
                        )

                    # Parallel drains to bf16 SBUF: ACT takes c[4:20],
                    # DVE takes c[0:4].
                    for r0, ps in ((0, ps_a), (3, ps_b)):
                        pv = (
                            ps[:, 0:1536]
                            .rearrange("p (r b) -> p r b", b=512)[:, :, 0:wn * W]
                            .rearrange("p r (w c) -> p r w c", c=W)
                        )
                        nc.scalar.copy(
                            out=ybu[:, r0:r0 + 3, w0:w0 + wn, :],
                            in_=pv[:, :, :, 4:20],
                        )
                        nc.vector.tensor_copy(
                            out=yc[:, r0:r0 + 3, w0:w0 + wn, :],
                            in_=pv[:, :, :, 0:4],
                        )
                    w0 += wn

                # Max tree in bf16 SBUF (DVE 2x mode; any pairing of positions
                # is valid for a max). Two half-chunk trees so the first
                # half's tree overlaps the second half's PSUM drains, and
                # bias+relu on DVE tensor_scalar (4x) to keep the ACT queue
                # free for the next chunk's drains.
                m_all = mpool.tile([128, 6, CHUNK_W], bf16, tag="m_all")
                HW2 = CHUNK_W // 2
                for hf in range(2):
                    w0 = hf * HW2
                    ybv = ybu[:, :, w0:w0 + HW2, :]
                    u1 = tpool.tile([128, 6, HW2, 8], bf16, tag="u1")
                    nc.vector.tensor_tensor(out=u1[:], in0=ybv[:, :, :, 0:8],
                                            in1=ybv[:, :, :, 8:16], op=ALU.max)
                    u2 = tpool.tile([128, 6, HW2, 4], bf16, tag="u2")
                    nc.vector.tensor_tensor(out=u2[:], in0=u1[:, :, :, 0:4],
                                            in1=u1[:, :, :, 4:8], op=ALU.max)
                    u3 = tpool.tile([128, 6, HW2, 4], bf16, tag="u3")
                    nc.vector.tensor_tensor(out=u3[:], in0=u2[:],
                                            in1=yc[:, :, w0:w0 + HW2, :],
                                            op=ALU.max)
                    u4 = tpool.tile([128, 6, HW2, 2], bf16, tag="u4")
                    nc.vector.tensor_tensor(out=u4[:], in0=u3[:, :, :, 0:2],
                                            in1=u3[:, :, :, 2:4], op=ALU.max)
                    m_pre = tpool.tile([128, 6, HW2], bf16, tag="m_pre")
                    nc.vector.tensor_tensor(out=m_pre[:], in0=u4[:, :, :, 0],
                                            in1=u4[:, :, :, 1], op=ALU.max)
                    for r in range(6):
                        nc.vector.tensor_scalar(
                            out=m_all[:, r, w0:w0 + HW2],
                            in0=m_pre[:, r, :],
                            scalar1=cbias_sb[:, r:r + 1],
                            scalar2=0.0,
                            op0=ALU.add,
                            op1=ALU.max,
                        )

                # Defer the linear layer by one chunk: chunk c-1's m_all is
                # long since ready, so the PE never stalls on the DVE tree /
                # ACT bias chain (its FIFO otherwise parks on this chunk's
                # linear matmuls while the next chunk's convs are ready).
                if m_prev is not None:
                    emit_linear(c - 1, m_prev)
                m_prev = m_all

            emit_linear(N_CHUNKS - 1, m_prev)

    nc.compile()
    return nc


def _prep_maps(chars, emb, w1, b1, w3, b3, w5, b5, lw, lb):
    flat = np.asarray(chars).reshape(-1, W).astype(np.int64)  # [8192, 20]
    emb = np.asarray(emb, dtype=np.float32)
    lw = np.asarray(lw, dtype=np.float32)
    convs = {1: np.asarray(w1, np.float32), 3: np.asarray(w3, np.float32),
             5: np.asarray(w5, np.float32)}
    biases = {1: np.asarray(b1, np.float32), 3: np.asarray(b3, np.float32),
              5: np.asarray(b5, np.float32)}

    rows = np.zeros((VOCAB + 1, 128), dtype=BF16)
    rows[:VOCAB, :E] = emb.astype(BF16)
    # SBUF-source gather layout (tpr=128, 256 B per rank): token v's row at
    # partition v & 127, bf16 elems [(v>>7)*128, +128).
    table = np.zeros((128, 3 * 128), dtype=BF16)
    for v in range(VOCAB + 1):
        r, tok = v >> 7, v & 127
        table[tok, r * 128:(r + 1) * 128] = rows[v]

    wconv = np.zeros((128, NQ * 128), dtype=BF16)
    for q, (k, hh, d) in enumerate(TAPS):
        wk = convs[k]  # [H, E, k]
        blk = np.zeros((128, 128), dtype=np.float32)
        blk[:E, :] = wk[hh * 128:(hh + 1) * 128, :, d].T
        if d + 1 < k:
            blk[E:, :] = wk[hh * 128:(hh + 1) * 128, :, d + 1].T
        wconv[:, q * 128:(q + 1) * 128] = blk.astype(BF16)

    lwt = np.zeros((128, 6 * 256), dtype=BF16)
    for r in range(6):
        lo, hi = LW_COLS[r]
        lwt[:, r * 256:(r + 1) * 256] = lw[:, lo:hi].T.astype(BF16)

    cbias = np.zeros((128, 6), dtype=np.float32)
    for r, (k, half) in enumerate([(5, 0), (5, 1), (3, 0), (3, 1), (1, 0), (1, 1)]):
        cbias[:, r] = biases[k][half * 128:(half + 1) * 128]

    lbias = np.asarray(lb, np.float32).reshape(1, 256).astype(BF16)

    in_maps = []
    for c in range(N_CORES):
        words = flat[c * NW:(c + 1) * NW]  # [NW, 20]
        padded = np.full((NW, WP), PAD_TOK, dtype=np.int16)
        padded[:, 2:2 + W] = words
        stream = padded.reshape(-1)  # [NW*22]
        wrapped = stream.reshape(-1, 16).T  # [16, IDX_COLS]
        idx = np.ascontiguousarray(np.tile(wrapped, (8, 1)))  # replicated x8
        in_maps.append({
            "idx": idx, "table": table, "wconv": wconv, "lwt": lwt,
            "cbias": cbias, "lbias": lbias,
        })
    return in_maps


_NC_CACHE = {}


def run(inputs, trace=False):
    if "nc" not in _NC_CACHE:
        _NC_CACHE["nc"] = _build_nc()
    nc = _NC_CACHE["nc"]
    in_maps = _prep_maps(**inputs)
    res = run_bass_kernel_spmd(nc, in_maps, list(range(N_CORES)), trace=trace)
    out = np.concatenate([res.results[i]["out"] for i in range(N_CORES)], axis=0)
    return out.reshape(B, S, H).astype(np.float32), res


def kernel(**inputs):
    out, _ = run(inputs)
    return out
